# revision 2
# baseline (speedup 1.0000x reference)
"""Trainium2 Bass kernel for nn_DSPEditor: per-frame label-driven mel editing.

Semantics (per sample, T frames, M=128 mel bins, labels in 0..7):
  CUT(0)/PAD(7) -> zero; LOOP(2) -> replay the min(seg_len, start) frames
  preceding the segment; FADE_IN(3)/FADE_OUT(4)/TRANSITION(6) -> per-frame
  gain; EFFECT(5) -> upper half bins x0.3; clip to [0,1].

Implementation (sparse gather -> scale -> scatter, bf16):
  Every output frame is scale_lo/hi[t] * mel[src[t]] where src and the two
  scales are pure functions of the (tiny) label tensor.  The host computes
  them exactly per the reference recurrences, then drops every frame whose
  output is exactly zero (cut/pad, fade-in frac==0, fade-out frac==1,
  loops sourcing cut/pad) - only ~60% of frames survive.  The mel payload
  is cast to bf16 (rel tol is 2e-2; bf16 rounding is ~4e-3 worst case).

  On device, per core (S samples), the surviving K rows flow through:
    dma_gather(gt <- melb[gidx])  ->  DVE mul by slot scales  ->
    dma_scatter_add(outb[sidx] += gt)
  in CH-slot chunks double-buffered across two SWDGE queues.  The output
  DRAM buffer is pre-zeroed by the runtime (run_bass_kernel_spmd contract,
  both native and PJRT-donation paths), so scatter-add == assignment and
  dropped rows are already correct.  The SPMD module is shared by all 8
  cores, so each core pads its keep-list to the fleet max with dummy slots
  (gather row 0, scale 0, scatter to one of its own dropped rows) and the
  per-chunk descriptor counts are baked in as constants.

Data parallel over 8 cores: 4 samples per core.
"""

import numpy as np
import ml_dtypes

import concourse.bass as bass
import concourse.bacc as bacc
import concourse.mybir as mybir
from concourse.tile import TileContext

F32 = mybir.dt.float32
BF16 = mybir.dt.bfloat16
I16 = mybir.dt.int16
AOP = mybir.AluOpType

P = 128          # partitions
M = 128          # mel bins


def build_bass(T, S, K_u, chunk=4096, reps=1, variant="all", dtype="bf16"):
    """Per-core Bass module.  T frames/sample, S samples/core, K_u kept
    slots (uniform across cores; host pads), chunk slots per pipeline step.

    reps > 1 replicates the body for steady-state throughput benching."""
    dt = BF16 if dtype == "bf16" else F32
    npdt = ml_dtypes.bfloat16 if dtype == "bf16" else np.float32
    nchunks = -(-K_u // chunk)
    K_pad = nchunks * chunk
    nc = bacc.Bacc("TRN2", target_bir_lowering=False, num_swdge_queues=2)

    melb = nc.dram_tensor("melb", [S * T, M], dt, kind="ExternalInput")
    gidx = nc.dram_tensor("gidx", [P, K_pad // 16], I16, kind="ExternalInput")
    sidx = nc.dram_tensor("sidx", [P, K_pad // 16], I16, kind="ExternalInput")
    slo = nc.dram_tensor("slo", [P, K_pad // P], dt, kind="ExternalInput")
    shi = nc.dram_tensor("shi", [P, K_pad // P], dt, kind="ExternalInput")
    outb = nc.dram_tensor("outb", [S * T, M], dt, kind="ExternalOutput")

    CW = chunk // 16     # idx cols per chunk
    CC = chunk // P      # slot cols per chunk

    with TileContext(nc) as tc:
        with (
            tc.tile_pool(name="sc", bufs=2) as sc,
            tc.tile_pool(name="mel", bufs=4) as mp,
        ):
            for _rep in range(reps):
                gi = sc.tile([P, K_pad // 16], I16, name="gi", tag="gi")
                nc.sync.dma_start(gi[:, :], gidx[:, :])
                si = sc.tile([P, K_pad // 16], I16, name="si", tag="si")
                nc.sync.dma_start(si[:, :], sidx[:, :])
                sl = sc.tile([P, K_pad // P], dt, name="sl", tag="sl")
                nc.sync.dma_start(sl[:, :], slo[:, :])
                sh = sc.tile([P, K_pad // P], dt, name="sh", tag="sh")
                nc.sync.dma_start(sh[:, :], shi[:, :])

                for k in range(nchunks):
                    nval = min(chunk, K_u - k * chunk)
                    gt = mp.tile([P, CC, M], dt, name="gt", tag="gt")
                    if variant == "scatter" and k > 0:
                        pass  # reuse previously gathered tile contents
                    else:
                        nc.gpsimd.dma_gather(
                            out_ap=gt[:, :, :], in_ap=melb[:, :],
                            idxs_ap=gi[:, k * CW:(k + 1) * CW],
                            num_idxs=chunk, num_idxs_reg=nval,
                            elem_size=M, single_packet=False, queue_num=0)
                    if variant != "nomul":
                        nc.vector.tensor_tensor(
                            out=gt[:, :, 0:M // 2], in0=gt[:, :, 0:M // 2],
                            in1=sl[:, k * CC:(k + 1) * CC].unsqueeze(2)
                                .to_broadcast([P, CC, M // 2]),
                            op=AOP.mult)
                        nc.vector.tensor_tensor(
                            out=gt[:, :, M // 2:M], in0=gt[:, :, M // 2:M],
                            in1=sh[:, k * CC:(k + 1) * CC].unsqueeze(2)
                                .to_broadcast([P, CC, M // 2]),
                            op=AOP.mult)
                    if variant == "gather" and k > 0:
                        continue  # skip the store side
                    nc.gpsimd.dma_scatter_add(
                        out_ap=outb[:, :], in_ap=gt[:, :, :],
                        idxs_ap=si[:, k * CW:(k + 1) * CW],
                        num_idxs=chunk, num_idxs_reg=nval,
                        elem_size=M, single_packet=False, queue_num=1)

    nc.finalize()
    return nc


def _edit_plan(edit_labels):
    """Vectorized replica of the reference per-frame recurrences.
    Returns (src, slo, shi, nz): source frame, lo/hi-half scales, and the
    output-is-nonzero mask, all (B, T)."""
    lab = np.asarray(edit_labels)
    Bt, T = lab.shape
    idx = np.arange(T)
    diff = lab[:, 1:] != lab[:, :-1]
    change = np.concatenate([np.ones((Bt, 1), bool), diff], 1)
    last = np.concatenate([diff, np.ones((Bt, 1), bool)], 1)
    start = np.maximum.accumulate(np.where(change, idx, 0), 1)
    end = np.flip(np.minimum.accumulate(
        np.flip(np.where(last, idx + 1, T), 1), 1), 1)
    seg = end - start
    pos = idx - start
    den = np.maximum(seg - 1, 1)
    frac = pos.astype(np.float32) / den.astype(np.float32)
    keep = ~((lab == 0) | (lab == 7))
    Lp = np.minimum(seg, start)
    src = start - Lp + np.where(Lp > 0, pos % np.maximum(Lp, 1), 0)
    src = np.clip(src, 0, T - 1)
    loop_mask = (lab == 2) & (Lp > 0)
    src = np.where(loop_mask, src, idx)
    ks = np.take_along_axis(keep, src, 1)
    gain = np.ones((Bt, T), np.float32)
    gain = np.where(lab == 3, frac, gain)
    gain = np.where(lab == 4, np.float32(1.0) - frac, gain)
    gain = np.where(lab == 6,
                    (1.0 - 0.5 * np.sin(np.pi * frac)).astype(np.float32),
                    gain)
    slo = np.where(ks, gain, np.float32(0.0))
    shi = slo * np.where(lab == 5, np.float32(0.3), np.float32(1.0))
    nz = slo != 0
    return src, slo, shi, nz


def make_inputs(raw_mel, edit_labels, n_cores=8, chunk=4096, dtype="bf16"):
    """Shard full inputs into per-core in_maps plus the build plan."""
    raw_mel = np.asarray(raw_mel)
    Bt, T, Mm = raw_mel.shape
    assert Mm == M
    S = Bt // n_cores
    npdt = ml_dtypes.bfloat16 if dtype == "bf16" else np.float32

    src, slo, shi, nz = _edit_plan(edit_labels)

    # per-core keep counts -> uniform K_u across the fleet
    nz_c = nz.reshape(n_cores, S * T)
    K_c = nz_c.sum(1)
    K_u = int(K_c.max())
    nchunks = -(-K_u // chunk)
    K_pad = nchunks * chunk

    soff = (np.arange(S) * T)[:, None]            # per-sample row offset
    src_flat = (src.reshape(n_cores, S, T) + soff[None]).reshape(
        n_cores, S * T)
    dst_flat = np.broadcast_to(
        (soff + np.arange(T)[None]).reshape(1, S * T), (n_cores, S * T))
    slo_c = slo.reshape(n_cores, S * T)
    shi_c = shi.reshape(n_cores, S * T)

    mel_t = raw_mel.astype(npdt)

    in_maps = []
    for c in range(n_cores):
        keep_pos = np.nonzero(nz_c[c])[0]
        K = keep_pos.size
        g = np.full(K_pad, -1, np.int32)
        s_ = np.full(K_pad, -1, np.int32)
        a = np.zeros(K_pad, np.float32)
        b = np.zeros(K_pad, np.float32)
        g[:K] = src_flat[c, keep_pos]
        s_[:K] = dst_flat[c, keep_pos]
        a[:K] = slo_c[c, keep_pos]
        b[:K] = shi_c[c, keep_pos]
        if K < K_u:
            # dummy slots: read row 0, scale 0, write one of this core's
            # dropped (hence all-zero, never-otherwise-written) rows
            drop = np.nonzero(~nz_c[c])[0]
            g[K:K_u] = 0
            s_[K:K_u] = dst_flat[c, drop[0]]
        gi = np.tile(np.ascontiguousarray(
            g.astype(np.int16).reshape(K_pad // 16, 16).T), (8, 1))
        si = np.tile(np.ascontiguousarray(
            s_.astype(np.int16).reshape(K_pad // 16, 16).T), (8, 1))
        sl = np.ascontiguousarray(
            a.reshape(K_pad // P, P).T.astype(npdt))
        sh = np.ascontiguousarray(
            b.reshape(K_pad // P, P).T.astype(npdt))
        in_maps.append({
            "melb": np.ascontiguousarray(
                mel_t[c * S:(c + 1) * S].reshape(S * T, M)),
            "gidx": gi,
            "sidx": si,
            "slo": sl,
            "shi": sh,
        })
    plan = dict(T=T, S=S, K_u=K_u, chunk=chunk, dtype=dtype)
    return in_maps, plan


_CACHE = {}


def _get_nc(plan):
    key = (plan["T"], plan["S"], plan["K_u"], plan["chunk"], plan["dtype"])
    if key not in _CACHE:
        _CACHE[key] = build_bass(plan["T"], plan["S"], plan["K_u"],
                                 chunk=plan["chunk"], dtype=plan["dtype"])
    return _CACHE[key]


def kernel(raw_mel, edit_labels):
    from concourse.bass_utils import run_bass_kernel_spmd

    raw_mel = np.asarray(raw_mel)
    edit_labels = np.asarray(edit_labels)
    Bt, T, Mm = raw_mel.shape
    n_cores = 8
    in_maps, plan = make_inputs(raw_mel, edit_labels, n_cores)
    nc = _get_nc(plan)
    res = run_bass_kernel_spmd(nc, in_maps, core_ids=list(range(n_cores)))
    S = plan["S"]
    out = np.concatenate(
        [np.asarray(r["outb"]).astype(np.float32).reshape(S, T, Mm)
         for r in res.results], axis=0)
    return out


# revision 3
# speedup vs baseline: 191.4445x; 191.4445x over previous
"""Trainium2 Bass kernel for nn_DSPEditor: per-frame label-driven mel editing.

Semantics (per sample, T frames, M=128 mel bins, labels in 0..7):
  CUT(0)/PAD(7) -> zero; LOOP(2) -> replay the min(seg_len, start) frames
  preceding the segment; FADE_IN(3)/FADE_OUT(4)/TRANSITION(6) -> per-frame
  gain; EFFECT(5) -> upper half bins x0.3; clip to [0,1].

Implementation (dense bf16 HWDGE pipeline + sparse loop patch):
  Every output frame is scale_lo/hi[t] * mel[src[t]], where src and the
  scales are cheap per-frame functions of the label tensor; the host
  computes them exactly per the reference recurrences (0.26 M labels —
  metadata only; all 134 MB of mel stays on device).  The mel payload is
  cast to bf16 (rel tol is 2e-2; bf16 rounding is ~5e-3 worst case).

  Per core (N = S*T frames, slot map f = p*C + c):
  1. LOOP pipeline (small): ant dma_gather fetches the ~3K loop-source
     rows (compacted lists, 4 SWDGE queues) -> DVE scale -> held in SBUF.
  2. DENSE pipeline (bulk): chunked HWDGE read of melb -> DVE scale with
     dense per-frame scales (cut/pad/fade-zero rows scale 0; loop rows
     scale 0, patched in step 3) -> HWDGE store to outb.  16.8 MB moves
     at HBM line rate with zero Q7 descriptors.
  3. ant dma_scatter_add adds the scaled loop rows into outb (the dense
     store wrote exact 0.0 there, so add == assignment); Tile's DRAM WAW
     tracking orders the scatters after the dense stores.  Scatter lists
     are binned into 4 disjoint row ranges on 4 queues.

  The SPMD module is shared by all 8 cores, so per-(core,bin) entry
  counts are padded to the fleet max with dummy slots (gather row 0,
  scale 0, scatter-ADD 0 into the range's row 0 - harmless).

  Sparse-descriptor alternatives (dma_gather for all rows, scatter-only
  output, indirect DMA) were measured descriptor-rate-bound (3.9-38
  ns/descriptor) and lose to the dense pipeline at ~400 GB/s/core.

Data parallel over 8 cores: 4 samples per core.  Correctness does not
depend on the runtime pre-zeroing the output buffer.
"""

import numpy as np
import ml_dtypes

import concourse.bass as bass
import concourse.bacc as bacc
import concourse.mybir as mybir
from concourse.tile import TileContext

F32 = mybir.dt.float32
BF16 = mybir.dt.bfloat16
I16 = mybir.dt.int16
AOP = mybir.AluOpType

P = 128
M = 128
NLQ = 4             # SWDGE queues for the loop pipeline
NBIN = 4            # disjoint output-row-range bins for loop scatters


def build_bass(T, S, U, SLK, nl, chunk_cols=32, reps=1, variant="all",
               dtype="bf16"):
    """U: uniform valid count per loop bin; SLK: slots per bin (multiple
    of 128, > U); nl: number of bins (0 = no loop frames anywhere).
    reps > 1 replicates the body for steady-state benchmarking."""
    dt = BF16 if dtype == "bf16" else F32
    N = S * T
    C = N // P
    nch = C // chunk_cols
    L_pad = nl * SLK
    nc = bacc.Bacc("TRN2", target_bir_lowering=False, num_swdge_queues=NLQ)

    melb = nc.dram_tensor("melb", [N, M], dt, kind="ExternalInput")
    sld = nc.dram_tensor("sld", [P, C], dt, kind="ExternalInput")
    shd = nc.dram_tensor("shd", [P, C], dt, kind="ExternalInput")
    if nl:
        lgi = nc.dram_tensor("lgi", [P, L_pad // 16], I16,
                             kind="ExternalInput")
        lsi = nc.dram_tensor("lsi", [P, L_pad // 16], I16,
                             kind="ExternalInput")
        lsl = nc.dram_tensor("lsl", [P, L_pad // P], dt, kind="ExternalInput")
        lsh = nc.dram_tensor("lsh", [P, L_pad // P], dt, kind="ExternalInput")
    outb = nc.dram_tensor("outb", [N, M], dt, kind="ExternalOutput")
    melv = melb.rearrange("(p c) m -> p c m", p=P)
    outv = outb.rearrange("(p c) m -> p c m", p=P)

    with TileContext(nc) as tc:
        with (
            tc.tile_pool(name="sc", bufs=2) as sc,
            tc.tile_pool(name="mel", bufs=4) as mp,
            tc.tile_pool(name="lp", bufs=max(2 * nl, 1)) as lp,
        ):
            for _rep in range(reps):
                sl = sc.tile([P, C], dt, name="sl", tag="sl")
                nc.sync.dma_start(sl[:, :], sld[:, :])
                sh = sc.tile([P, C], dt, name="sh", tag="sh")
                nc.sync.dma_start(sh[:, :], shd[:, :])
                do_loop = nl and variant in ("all", "looponly")
                if do_loop:
                    lgt = sc.tile([P, L_pad // 16], I16, name="lgt", tag="lgt")
                    nc.sync.dma_start(lgt[:, :], lgi[:, :])
                    lst = sc.tile([P, L_pad // 16], I16, name="lst", tag="lst")
                    nc.sync.dma_start(lst[:, :], lsi[:, :])
                    lslt = sc.tile([P, L_pad // P], dt, name="lslt",
                                   tag="lslt")
                    nc.sync.dma_start(lslt[:, :], lsl[:, :])
                    lsht = sc.tile([P, L_pad // P], dt, name="lsht",
                                   tag="lsht")
                    nc.sync.dma_start(lsht[:, :], lsh[:, :])
                    lts = []
                    for j in range(nl):
                        CW = SLK // 16
                        CL = SLK // P
                        lt = lp.tile([P, CL, M], dt, name=f"lt{j}",
                                     tag=f"lt{j}")
                        nc.gpsimd.dma_gather(
                            out_ap=lt[:, :, :], in_ap=melb[:, :],
                            idxs_ap=lgt[:, j * CW:(j + 1) * CW],
                            num_idxs=SLK, num_idxs_reg=U,
                            elem_size=M, single_packet=False,
                            queue_num=j % NLQ)
                        nc.vector.tensor_tensor(
                            out=lt[:, :, 0:M // 2], in0=lt[:, :, 0:M // 2],
                            in1=lslt[:, j * CL:(j + 1) * CL].unsqueeze(2)
                                .to_broadcast([P, CL, M // 2]),
                            op=AOP.mult)
                        nc.vector.tensor_tensor(
                            out=lt[:, :, M // 2:M], in0=lt[:, :, M // 2:M],
                            in1=lsht[:, j * CL:(j + 1) * CL].unsqueeze(2)
                                .to_broadcast([P, CL, M // 2]),
                            op=AOP.mult)
                        lts.append(lt)

                if variant in ("all", "denseonly"):
                    for k in range(nch):
                        cs = slice(k * chunk_cols, (k + 1) * chunk_cols)
                        gt = mp.tile([P, chunk_cols, M], dt,
                                     name="gt", tag="gt")
                        nc.sync.dma_start(gt[:, :, :], melv[:, cs, :])
                        nc.vector.tensor_tensor(
                            out=gt[:, :, 0:M // 2], in0=gt[:, :, 0:M // 2],
                            in1=sl[:, cs].unsqueeze(2)
                                .to_broadcast([P, chunk_cols, M // 2]),
                            op=AOP.mult)
                        nc.vector.tensor_tensor(
                            out=gt[:, :, M // 2:M], in0=gt[:, :, M // 2:M],
                            in1=sh[:, cs].unsqueeze(2)
                                .to_broadcast([P, chunk_cols, M // 2]),
                            op=AOP.mult)
                        nc.sync.dma_start(outv[:, cs, :], gt[:, :, :])

                if do_loop:
                    R = N // nl
                    for j in range(nl):
                        CW = SLK // 16
                        nc.gpsimd.dma_scatter_add(
                            out_ap=outb[j * R:(j + 1) * R, :],
                            in_ap=lts[j][:, :, :],
                            idxs_ap=lst[:, j * CW:(j + 1) * CW],
                            num_idxs=SLK, num_idxs_reg=U,
                            elem_size=M, single_packet=False,
                            queue_num=j % NLQ)

    nc.finalize()
    return nc


def _edit_plan(edit_labels):
    """Vectorized replica of the reference per-frame recurrences.
    Returns (src, slo, shi, nz): source frame, lo/hi-half scales, and the
    output-is-nonzero mask, all (B, T)."""
    lab = np.asarray(edit_labels)
    Bt, T = lab.shape
    idx = np.arange(T)
    diff = lab[:, 1:] != lab[:, :-1]
    change = np.concatenate([np.ones((Bt, 1), bool), diff], 1)
    last = np.concatenate([diff, np.ones((Bt, 1), bool)], 1)
    start = np.maximum.accumulate(np.where(change, idx, 0), 1)
    end = np.flip(np.minimum.accumulate(
        np.flip(np.where(last, idx + 1, T), 1), 1), 1)
    seg = end - start
    pos = idx - start
    den = np.maximum(seg - 1, 1)
    frac = pos.astype(np.float32) / den.astype(np.float32)
    keep = ~((lab == 0) | (lab == 7))
    Lp = np.minimum(seg, start)
    src = start - Lp + np.where(Lp > 0, pos % np.maximum(Lp, 1), 0)
    src = np.clip(src, 0, T - 1)
    loop_mask = (lab == 2) & (Lp > 0)
    src = np.where(loop_mask, src, idx)
    ks = np.take_along_axis(keep, src, 1)
    gain = np.ones((Bt, T), np.float32)
    gain = np.where(lab == 3, frac, gain)
    gain = np.where(lab == 4, np.float32(1.0) - frac, gain)
    gain = np.where(lab == 6,
                    (1.0 - 0.5 * np.sin(np.pi * frac)).astype(np.float32),
                    gain)
    slo = np.where(ks, gain, np.float32(0.0))
    shi = slo * np.where(lab == 5, np.float32(0.3), np.float32(1.0))
    nz = slo != 0
    return src, slo, shi, nz


def make_inputs(raw_mel, edit_labels, n_cores=8, dtype="bf16"):
    """Shard full inputs into per-core in_maps plus the build plan."""
    raw_mel = np.asarray(raw_mel)
    Bt, T, Mm = raw_mel.shape
    assert Mm == M
    S = Bt // n_cores
    N = S * T
    C = N // P
    npdt = ml_dtypes.bfloat16 if dtype == "bf16" else np.float32

    src, slo, shi, nz = _edit_plan(edit_labels)
    lab = np.asarray(edit_labels)
    loopk = (lab == 2) & (src != np.arange(T)[None, :]) & nz

    soff = (np.arange(S) * T)[:, None]
    src_f = (src.reshape(n_cores, S, T) + soff[None]).reshape(n_cores, N)
    loop_c = loopk.reshape(n_cores, N)
    nz_c = nz.reshape(n_cores, N)
    slo_c = slo.reshape(n_cores, N)
    shi_c = shi.reshape(n_cores, N)

    nl = NBIN
    R = N // nl
    U = 0
    for c in range(n_cores):
        pos = np.nonzero(loop_c[c])[0]
        for j in range(nl):
            U = max(U, int(((pos >= j * R) & (pos < (j + 1) * R)).sum()))
    if U == 0:
        nl = 0
    SLK = -(-(U + 1) // P) * P if U else 0
    L_pad = nl * SLK

    mel_t = raw_mel.astype(npdt)

    in_maps = []
    for c in range(n_cores):
        dense_keep = nz_c[c] & ~loop_c[c]
        a = np.where(dense_keep, slo_c[c], 0).astype(np.float32)
        b = np.where(dense_keep, shi_c[c], 0).astype(np.float32)
        im = {
            "melb": np.ascontiguousarray(
                mel_t[c * S:(c + 1) * S].reshape(N, M)),
            "sld": np.ascontiguousarray(a.astype(npdt).reshape(P, C)),
            "shd": np.ascontiguousarray(b.astype(npdt).reshape(P, C)),
        }
        if nl:
            g = np.full(L_pad, -1, np.int32)
            s_ = np.full(L_pad, -1, np.int32)
            la = np.zeros(L_pad, np.float32)
            lb = np.zeros(L_pad, np.float32)
            pos = np.nonzero(loop_c[c])[0]
            for j in range(nl):
                pj = pos[(pos >= j * R) & (pos < (j + 1) * R)]
                Lj = pj.size
                o = j * SLK
                g[o:o + Lj] = src_f[c, pj]
                s_[o:o + Lj] = pj - j * R
                la[o:o + Lj] = slo_c[c, pj]
                lb[o:o + Lj] = shi_c[c, pj]
                if Lj < U:
                    # dummies: gather row 0, scale 0 -> scatter-ADDs an
                    # exact 0 into the range's row 0 (harmless)
                    g[o + Lj:o + U] = 0
                    s_[o + Lj:o + U] = 0
            im["lgi"] = np.tile(np.ascontiguousarray(
                g.astype(np.int16).reshape(L_pad // 16, 16).T), (8, 1))
            im["lsi"] = np.tile(np.ascontiguousarray(
                s_.astype(np.int16).reshape(L_pad // 16, 16).T), (8, 1))
            im["lsl"] = np.ascontiguousarray(
                la.reshape(L_pad // P, P).T.astype(npdt))
            im["lsh"] = np.ascontiguousarray(
                lb.reshape(L_pad // P, P).T.astype(npdt))
        in_maps.append(im)
    plan = dict(T=T, S=S, U=U, SLK=SLK, nl=nl, dtype=dtype)
    return in_maps, plan


_CACHE = {}


def _get_nc(plan):
    key = tuple(sorted(plan.items()))
    if key not in _CACHE:
        _CACHE[key] = build_bass(plan["T"], plan["S"], plan["U"],
                                 plan["SLK"], plan["nl"],
                                 dtype=plan["dtype"])
    return _CACHE[key]


def kernel(raw_mel, edit_labels):
    from concourse.bass_utils import run_bass_kernel_spmd

    raw_mel = np.asarray(raw_mel)
    edit_labels = np.asarray(edit_labels)
    Bt, T, Mm = raw_mel.shape
    n_cores = 8
    in_maps, plan = make_inputs(raw_mel, edit_labels, n_cores)
    nc = _get_nc(plan)
    res = run_bass_kernel_spmd(nc, in_maps, core_ids=list(range(n_cores)))
    S = plan["S"]
    out = np.concatenate(
        [np.asarray(r["outb"]).astype(np.float32).reshape(S, T, Mm)
         for r in res.results], axis=0)
    return out


# revision 7
# speedup vs baseline: 207.2821x; 1.0827x over previous
"""Trainium2 Bass kernel for nn_DSPEditor: per-frame label-driven mel editing.

Semantics (per sample, T frames, M=128 mel bins, labels in 0..7):
  CUT(0)/PAD(7) -> zero; LOOP(2) -> replay the min(seg_len, start) frames
  preceding the segment; FADE_IN(3)/FADE_OUT(4)/TRANSITION(6) -> per-frame
  gain; EFFECT(5) -> upper half bins x0.3; clip to [0,1].

Implementation (dense bf16 HWDGE pipeline + sparse loop patch):
  Every output frame is scale_lo/hi[t] * mel[src[t]], where src and the
  scales are cheap per-frame functions of the label tensor; the host
  computes them exactly per the reference recurrences (0.26 M labels —
  metadata only; all 134 MB of mel stays on device).  The mel payload is
  cast to bf16 (rel tol is 2e-2; bf16 rounding is ~5e-3 worst case).

  Per core (N = S*T frames, slot map f = p*C + c):
  1. LOOP pipeline (small): ant dma_gather fetches the ~3K loop-source
     rows (compacted lists, 4 SWDGE queues) -> DVE scale -> held in SBUF.
  2. DENSE pipeline (bulk): chunked HWDGE read of melb -> DVE scale with
     dense per-frame scales (cut/pad/fade-zero rows scale 0; loop rows
     scale 0, patched in step 3) -> HWDGE store to outb.  16.8 MB moves
     at HBM line rate with zero Q7 descriptors.
  3. ant dma_scatter_add adds the scaled loop rows into outb (the dense
     store wrote exact 0.0 there, so add == assignment); Tile's DRAM WAW
     tracking orders the scatters after the dense stores.  Scatter lists
     are binned into 4 disjoint row ranges on 4 queues.

  The SPMD module is shared by all 8 cores, so per-(core,bin) entry
  counts are padded to the fleet max with dummy slots (gather row 0,
  scale 0, scatter-ADD 0 into the range's row 0 - harmless).

  Sparse-descriptor alternatives (dma_gather for all rows, scatter-only
  output, indirect DMA) were measured descriptor-rate-bound (3.9-38
  ns/descriptor) and lose to the dense pipeline at ~400 GB/s/core.

Data parallel over 8 cores: 4 samples per core.  Correctness does not
depend on the runtime pre-zeroing the output buffer.
"""

import numpy as np
import ml_dtypes

import concourse.bass as bass
import concourse.bacc as bacc
import concourse.mybir as mybir
from concourse.tile import TileContext

F32 = mybir.dt.float32
BF16 = mybir.dt.bfloat16
I16 = mybir.dt.int16
AOP = mybir.AluOpType

P = 128
M = 128
NLQ = 4             # SWDGE queues for the loop pipeline
NBIN = 4            # disjoint output-row-range bins for loop scatters


def build_bass(T, S, U, SLK, nl, chunk_cols=32, reps=1, variant="all",
               dtype="bf16"):
    """U: uniform valid count per loop bin; SLK: slots per bin (multiple
    of 128, > U); nl: number of bins (0 = no loop frames anywhere).
    reps > 1 replicates the body for steady-state benchmarking."""
    dt = BF16 if dtype == "bf16" else F32
    N = S * T
    C = N // P
    nch = C // chunk_cols
    L_pad = nl * SLK
    nc = bacc.Bacc("TRN2", target_bir_lowering=False, num_swdge_queues=NLQ)

    melb = nc.dram_tensor("melb", [N, M], dt, kind="ExternalInput")
    sld = nc.dram_tensor("sld", [P, C], dt, kind="ExternalInput")
    shd = nc.dram_tensor("shd", [P, C], dt, kind="ExternalInput")
    if nl:
        lgi = nc.dram_tensor("lgi", [P, L_pad // 16], I16,
                             kind="ExternalInput")
        lsi = nc.dram_tensor("lsi", [P, L_pad // 16], I16,
                             kind="ExternalInput")
        lsl = nc.dram_tensor("lsl", [P, L_pad // P], dt, kind="ExternalInput")
        lsh = nc.dram_tensor("lsh", [P, L_pad // P], dt, kind="ExternalInput")
    outb = nc.dram_tensor("outb", [N, M], dt, kind="ExternalOutput")
    melv = melb.rearrange("(p c) m -> p c m", p=P)
    outv = outb.rearrange("(p c) m -> p c m", p=P)

    with TileContext(nc) as tc:
        with (
            tc.tile_pool(name="sc", bufs=2) as sc,
            tc.tile_pool(name="mel", bufs=4) as mp,
            tc.tile_pool(name="lp", bufs=max(2 * nl, 1)) as lp,
        ):
            psems = [nc.alloc_semaphore(f"lscq{q}")
                     for q in range(min(nl, NLQ))] if nl else []
            for _rep in range(reps):
                sl = sc.tile([P, C], dt, name="sl", tag="sl")
                nc.sync.dma_start(sl[:, :], sld[:, :])
                sh = sc.tile([P, C], dt, name="sh", tag="sh")
                nc.sync.dma_start(sh[:, :], shd[:, :])
                do_loop = nl and variant in ("all", "looponly")
                if do_loop:
                    lgt = sc.tile([P, L_pad // 16], I16, name="lgt", tag="lgt")
                    nc.sync.dma_start(lgt[:, :], lgi[:, :])
                    lst = sc.tile([P, L_pad // 16], I16, name="lst", tag="lst")
                    nc.sync.dma_start(lst[:, :], lsi[:, :])
                    lslt = sc.tile([P, L_pad // P], dt, name="lslt",
                                   tag="lslt")
                    nc.sync.dma_start(lslt[:, :], lsl[:, :])
                    lsht = sc.tile([P, L_pad // P], dt, name="lsht",
                                   tag="lsht")
                    nc.sync.dma_start(lsht[:, :], lsh[:, :])
                    lts = []
                    for j in range(nl):
                        CW = SLK // 16
                        CL = SLK // P
                        lt = lp.tile([P, CL, M], dt, name=f"lt{j}",
                                     tag=f"lt{j}")
                        nc.gpsimd.dma_gather(
                            out_ap=lt[:, :, :], in_ap=melb[:, :],
                            idxs_ap=lgt[:, j * CW:(j + 1) * CW],
                            num_idxs=SLK, num_idxs_reg=U,
                            elem_size=M, single_packet=False,
                            queue_num=j % NLQ)
                        nc.vector.tensor_tensor(
                            out=lt[:, :, 0:M // 2], in0=lt[:, :, 0:M // 2],
                            in1=lslt[:, j * CL:(j + 1) * CL].unsqueeze(2)
                                .to_broadcast([P, CL, M // 2]),
                            op=AOP.mult)
                        nc.vector.tensor_tensor(
                            out=lt[:, :, M // 2:M], in0=lt[:, :, M // 2:M],
                            in1=lsht[:, j * CL:(j + 1) * CL].unsqueeze(2)
                                .to_broadcast([P, CL, M // 2]),
                            op=AOP.mult)
                        lts.append(lt)

                # scatter PREPS: Q7 generates the descriptors now (they
                # overlap the dense phase); the DMAs fire at the triggers
                # below, after the last dense store
                preps = []
                if do_loop and variant == "all":
                    R = N // nl
                    for j in range(nl):
                        CW = SLK // 16
                        preps.append(nc.gpsimd.dma_scatter_add(
                            out_ap=outb[j * R:(j + 1) * R, :],
                            in_ap=lts[j][:, :, :],
                            idxs_ap=lst[:, j * CW:(j + 1) * CW],
                            num_idxs=SLK, num_idxs_reg=U,
                            elem_size=M, single_packet=False,
                            queue_num=j % NLQ,
                            prepare_only=True, sem=psems[j % NLQ]))

                if variant in ("all", "denseonly"):
                    for k in range(nch):
                        cs = slice(k * chunk_cols, (k + 1) * chunk_cols)
                        gt = mp.tile([P, chunk_cols, M], dt,
                                     name="gt", tag="gt")
                        nc.sync.dma_start(gt[:, :, :], melv[:, cs, :])
                        nc.vector.tensor_tensor(
                            out=gt[:, :, 0:M // 2], in0=gt[:, :, 0:M // 2],
                            in1=sl[:, cs].unsqueeze(2)
                                .to_broadcast([P, chunk_cols, M // 2]),
                            op=AOP.mult)
                        nc.vector.tensor_tensor(
                            out=gt[:, :, M // 2:M], in0=gt[:, :, M // 2:M],
                            in1=sh[:, cs].unsqueeze(2)
                                .to_broadcast([P, chunk_cols, M // 2]),
                            op=AOP.mult)
                        st = nc.sync.dma_start(outv[:, cs, :], gt[:, :, :])
                        for pi in preps:
                            # prep's dst write fires at trigger time (after
                            # this store); drop the circular WAW edge
                            st.ins.try_remove_dependency(pi.ins.name)

                if do_loop and variant == "all":
                    from concourse.instruction_name_ordered_set \
                        import InstructionNameOrderedSet
                    for q in range(min(nl, NLQ)):
                        tr = nc.gpsimd.trigger_dma(count=None, queue_num=q)
                        dep = InstructionNameOrderedSet()
                        dep.add(st.ins.name)   # last dense store
                        tr.ins.add_sync_dependencies_from(dep)
                elif do_loop:
                    R = N // nl
                    for j in range(nl):
                        CW = SLK // 16
                        nc.gpsimd.dma_scatter_add(
                            out_ap=outb[j * R:(j + 1) * R, :],
                            in_ap=lts[j][:, :, :],
                            idxs_ap=lst[:, j * CW:(j + 1) * CW],
                            num_idxs=SLK, num_idxs_reg=U,
                            elem_size=M, single_packet=False,
                            queue_num=j % NLQ)

    nc.finalize()
    return nc


def _edit_plan(edit_labels):
    """Vectorized replica of the reference per-frame recurrences.
    Returns (src, slo, shi, nz): source frame, lo/hi-half scales, and the
    output-is-nonzero mask, all (B, T)."""
    lab = np.asarray(edit_labels)
    Bt, T = lab.shape
    idx = np.arange(T)
    diff = lab[:, 1:] != lab[:, :-1]
    change = np.concatenate([np.ones((Bt, 1), bool), diff], 1)
    last = np.concatenate([diff, np.ones((Bt, 1), bool)], 1)
    start = np.maximum.accumulate(np.where(change, idx, 0), 1)
    end = np.flip(np.minimum.accumulate(
        np.flip(np.where(last, idx + 1, T), 1), 1), 1)
    seg = end - start
    pos = idx - start
    den = np.maximum(seg - 1, 1)
    frac = pos.astype(np.float32) / den.astype(np.float32)
    keep = ~((lab == 0) | (lab == 7))
    Lp = np.minimum(seg, start)
    src = start - Lp + np.where(Lp > 0, pos % np.maximum(Lp, 1), 0)
    src = np.clip(src, 0, T - 1)
    loop_mask = (lab == 2) & (Lp > 0)
    src = np.where(loop_mask, src, idx)
    ks = np.take_along_axis(keep, src, 1)
    gain = np.ones((Bt, T), np.float32)
    gain = np.where(lab == 3, frac, gain)
    gain = np.where(lab == 4, np.float32(1.0) - frac, gain)
    gain = np.where(lab == 6,
                    (1.0 - 0.5 * np.sin(np.pi * frac)).astype(np.float32),
                    gain)
    slo = np.where(ks, gain, np.float32(0.0))
    shi = slo * np.where(lab == 5, np.float32(0.3), np.float32(1.0))
    nz = slo != 0
    return src, slo, shi, nz


def make_inputs(raw_mel, edit_labels, n_cores=8, dtype="bf16"):
    """Shard full inputs into per-core in_maps plus the build plan."""
    raw_mel = np.asarray(raw_mel)
    Bt, T, Mm = raw_mel.shape
    assert Mm == M
    S = Bt // n_cores
    N = S * T
    C = N // P
    npdt = ml_dtypes.bfloat16 if dtype == "bf16" else np.float32

    src, slo, shi, nz = _edit_plan(edit_labels)
    lab = np.asarray(edit_labels)
    loopk = (lab == 2) & (src != np.arange(T)[None, :]) & nz

    soff = (np.arange(S) * T)[:, None]
    src_f = (src.reshape(n_cores, S, T) + soff[None]).reshape(n_cores, N)
    loop_c = loopk.reshape(n_cores, N)
    nz_c = nz.reshape(n_cores, N)
    slo_c = slo.reshape(n_cores, N)
    shi_c = shi.reshape(n_cores, N)

    nl = NBIN
    R = N // nl
    U = 0
    for c in range(n_cores):
        pos = np.nonzero(loop_c[c])[0]
        for j in range(nl):
            U = max(U, int(((pos >= j * R) & (pos < (j + 1) * R)).sum()))
    if U == 0:
        nl = 0
    SLK = -(-(U + 1) // P) * P if U else 0
    L_pad = nl * SLK

    mel_t = raw_mel.astype(npdt)

    in_maps = []
    for c in range(n_cores):
        dense_keep = nz_c[c] & ~loop_c[c]
        a = np.where(dense_keep, slo_c[c], 0).astype(np.float32)
        b = np.where(dense_keep, shi_c[c], 0).astype(np.float32)
        im = {
            "melb": np.ascontiguousarray(
                mel_t[c * S:(c + 1) * S].reshape(N, M)),
            "sld": np.ascontiguousarray(a.astype(npdt).reshape(P, C)),
            "shd": np.ascontiguousarray(b.astype(npdt).reshape(P, C)),
        }
        if nl:
            g = np.full(L_pad, -1, np.int32)
            s_ = np.full(L_pad, -1, np.int32)
            la = np.zeros(L_pad, np.float32)
            lb = np.zeros(L_pad, np.float32)
            pos = np.nonzero(loop_c[c])[0]
            for j in range(nl):
                pj = pos[(pos >= j * R) & (pos < (j + 1) * R)]
                Lj = pj.size
                o = j * SLK
                g[o:o + Lj] = src_f[c, pj]
                s_[o:o + Lj] = pj - j * R
                la[o:o + Lj] = slo_c[c, pj]
                lb[o:o + Lj] = shi_c[c, pj]
                if Lj < U:
                    # dummies: gather row 0, scale 0 -> scatter-ADDs an
                    # exact 0 into the range's row 0 (harmless)
                    g[o + Lj:o + U] = 0
                    s_[o + Lj:o + U] = 0
            im["lgi"] = np.tile(np.ascontiguousarray(
                g.astype(np.int16).reshape(L_pad // 16, 16).T), (8, 1))
            im["lsi"] = np.tile(np.ascontiguousarray(
                s_.astype(np.int16).reshape(L_pad // 16, 16).T), (8, 1))
            im["lsl"] = np.ascontiguousarray(
                la.reshape(L_pad // P, P).T.astype(npdt))
            im["lsh"] = np.ascontiguousarray(
                lb.reshape(L_pad // P, P).T.astype(npdt))
        in_maps.append(im)
    plan = dict(T=T, S=S, U=U, SLK=SLK, nl=nl, dtype=dtype)
    return in_maps, plan


_CACHE = {}


def _get_nc(plan):
    key = tuple(sorted(plan.items()))
    if key not in _CACHE:
        _CACHE[key] = build_bass(plan["T"], plan["S"], plan["U"],
                                 plan["SLK"], plan["nl"],
                                 dtype=plan["dtype"])
    return _CACHE[key]


def kernel(raw_mel, edit_labels):
    from concourse.bass_utils import run_bass_kernel_spmd

    raw_mel = np.asarray(raw_mel)
    edit_labels = np.asarray(edit_labels)
    Bt, T, Mm = raw_mel.shape
    n_cores = 8
    in_maps, plan = make_inputs(raw_mel, edit_labels, n_cores)
    nc = _get_nc(plan)
    res = run_bass_kernel_spmd(nc, in_maps, core_ids=list(range(n_cores)))
    S = plan["S"]
    out = np.concatenate(
        [np.asarray(r["outb"]).astype(np.float32).reshape(S, T, Mm)
         for r in res.results], axis=0)
    return out
